# revision 5
# baseline (speedup 1.0000x reference)
"""CVAE loss kernel for Trainium2 (8 NeuronCores, data-parallel over batch).

Strategy:
  - Sort samples by sequence length (host, O(B log B)) and deal them
    round-robin to the 8 cores so every core gets an identical length
    profile. Within a core, samples are length-sorted, so each of the 4
    partition-blocks of 128 samples only needs its own max length Wb of
    the 1024 timesteps: DMA and compute shrink to ~62% of the dense work.
  - Samples live on SBUF partitions, the (W*F) row of each sample on the
    free dimension. All O(B*L*F) work happens on-device with fused masked
    reductions (scalar_tensor_tensor / tensor_scalar / activation with
    accum_out).
  - Per-sample partial sums (one column per loss term) are DMA'd back; the
    host does only O(B) finishing math (per-sample normalization, endpoint
    gathers, the final weighted scalar sum).
"""

import numpy as np

import concourse.bacc as bacc
import concourse.tile as tile
from concourse import mybir, bass_utils

# Problem constants (hardcoded per contest rules).
B, L, F = 4096, 1024, 5
LATENT = 128
NCORES = 8
SPC = B // NCORES          # samples per core = 512
NBLK = SPC // 128          # partition blocks per core = 4
PI = float(np.pi)

# loss weights (match CVAELoss defaults)
W_POS, W_TIME, W_DIR, W_EP, W_LEN = 3.0, 0.5, 3.0, 10.0, 2.0
W_SPD, W_DECEL, W_DSM, W_TSM, W_KL = 1.5, 2.0, 2.5, 3.0, 0.01

OP = mybir.AluOpType
AF = mybir.ActivationFunctionType
DT = mybir.dt

# partials columns
C_MSUM, C_Q0, C_Q1, C_Q2, C_Q4, C_DIR, C_DECEL, C_DSM, C_TSM, C_LVE, C_ELV, C_MU2 = range(12)
NCOL = 12

_CACHE = {}


def _build_nc(ws):
    """ws: per-block free-dim widths (max sequence length in each block)."""
    nc = bacc.Bacc("TRN2", target_bir_lowering=False, debug=False)
    rts = [(nc.dram_tensor(f"r{b}", [128, ws[b] * F], DT.float32, kind="ExternalInput"),
            nc.dram_tensor(f"t{b}", [128, ws[b] * F], DT.float32, kind="ExternalInput"))
           for b in range(NBLK)]
    mu = nc.dram_tensor("mu", [SPC, LATENT], DT.float32, kind="ExternalInput")
    lv = nc.dram_tensor("lv", [SPC, LATENT], DT.float32, kind="ExternalInput")
    lens = nc.dram_tensor("lens", [SPC, 1], DT.float32, kind="ExternalInput")
    out = nc.dram_tensor("out", [SPC, NCOL], DT.float32, kind="ExternalOutput")

    muv = mu.ap().rearrange("(b p) d -> b p d", p=128)
    lvv = lv.ap().rearrange("(b p) d -> b p d", p=128)
    lensv = lens.ap().rearrange("(b p) d -> b p d", p=128)
    outv = out.ap().rearrange("(b p) d -> b p d", p=128)

    with tile.TileContext(nc) as tc:
        with (
            tc.tile_pool(name="io", bufs=2) as io,          # big R/T tiles
            tc.tile_pool(name="tmp", bufs=2) as tmp,        # f32 scratch
            tc.tile_pool(name="tmpb", bufs=2) as tmpb,      # bf16 scratch
            tc.tile_pool(name="keep", bufs=NBLK) as keep,   # persists across phases
            tc.tile_pool(name="one", bufs=1) as one,        # constants
        ):
            # constants
            it32 = one.tile([128, L], DT.int32)
            nc.gpsimd.iota(it32, pattern=[[1, L]], base=0, channel_multiplier=0)
            itf = one.tile([128, L], DT.float32)
            nc.vector.tensor_copy(itf, it32)
            epsb = one.tile([128, 1], DT.float32)
            nc.vector.memset(epsb, 1e-8)

            sts, mss, lts = [], [], []
            # ---------------- main block loop (trig_and_small ACT set) --------
            for b in range(NBLK):
                W = ws[b]
                rt = io.tile([128, W * F], DT.float32, tag="rt")
                tt = io.tile([128, W * F], DT.float32, tag="tt")
                nc.sync.dma_start(out=rt, in_=rts[b][0].ap())
                nc.sync.dma_start(out=tt, in_=rts[b][1].ap())
                lt = keep.tile([128, 1], DT.float32, tag="lt")
                nc.sync.dma_start(out=lt, in_=lensv[b])
                st = keep.tile([128, NCOL], DT.float32, tag="st")

                # valid mask (bf16) + its row-sum (msum)
                mv = tmp.tile([128, W], DT.bfloat16, tag="mv")
                nc.vector.tensor_scalar(out=mv, in0=itf[:, :W], scalar1=lt[:, 0:1],
                                        scalar2=None, op0=OP.is_lt, op1=OP.add,
                                        accum_out=st[:, C_MSUM:C_MSUM + 1])

                # --- masked squared diffs for features 0,1,2,4 ---
                for f, col in ((0, C_Q0), (1, C_Q1), (2, C_Q2), (4, C_Q4)):
                    d = tmpb.tile([128, W], DT.bfloat16, tag="d")
                    nc.vector.tensor_tensor(out=d, in0=rt[:, f::F], in1=tt[:, f::F],
                                            op=OP.subtract)
                    dm = tmpb.tile([128, W], DT.bfloat16, tag="dm")
                    nc.vector.tensor_tensor(out=dm, in0=d, in1=mv, op=OP.mult)
                    sq = tmpb.tile([128, W], DT.bfloat16, tag="sq")
                    nc.scalar.activation(out=sq, in_=dm, func=AF.Square, scale=1.0,
                                         accum_out=st[:, col:col + 1])

                # --- direction loss: 1-cos(pi*d3) = 2*sin^2(pi*wrap(d3/2)) ---
                d3 = tmp.tile([128, W], DT.float32, tag="d3")
                nc.vector.tensor_tensor(out=d3, in0=rt[:, 3::F], in1=tt[:, 3::F],
                                        op=OP.subtract)
                dm3h = tmp.tile([128, W], DT.float32, tag="dmh")
                nc.vector.scalar_tensor_tensor(out=dm3h, in0=d3, scalar=0.5, in1=mv,
                                               op0=OP.mult, op1=OP.mult)
                k32 = tmp.tile([128, W], DT.int32, tag="k32")
                nc.vector.tensor_copy(k32, dm3h)
                kf = tmp.tile([128, W], DT.float32, tag="kf")
                nc.vector.tensor_copy(kf, k32)
                u = tmp.tile([128, W], DT.float32, tag="u")
                nc.vector.tensor_tensor(out=u, in0=dm3h, in1=kf, op=OP.subtract)
                s = tmp.tile([128, W], DT.float32, tag="s")
                nc.scalar.activation(out=s, in_=u, func=AF.Sin, scale=PI)
                sq3 = tmpb.tile([128, W], DT.bfloat16, tag="sq")
                nc.scalar.activation(out=sq3, in_=s, func=AF.Square, scale=1.0,
                                     accum_out=st[:, C_DIR:C_DIR + 1])

                # --- speed deceleration: sum relu(s[l+1]-s[l]) * dmask ---
                sdiff = tmp.tile([128, W - 1], DT.float32, tag="sdiff")
                nc.vector.tensor_tensor(out=sdiff, in0=rt[:, F + 4::F],
                                        in1=rt[:, 4:(W - 1) * F:F], op=OP.subtract)
                jk1 = tmpb.tile([128, W - 1], DT.bfloat16, tag="jk")
                nc.vector.scalar_tensor_tensor(out=jk1, in0=sdiff, scalar=0.0,
                                               in1=mv[:, 1:], op0=OP.max, op1=OP.mult,
                                               accum_out=st[:, C_DECEL:C_DECEL + 1])

                # --- direction smoothness: wrapped(pi*(d[l+1]-d[l]))^2 * dmask ---
                dd = tmp.tile([128, W - 1], DT.float32, tag="sdiff")
                nc.vector.tensor_tensor(out=dd, in0=rt[:, F + 3::F],
                                        in1=rt[:, 3:(W - 1) * F:F], op=OP.subtract)
                ddh = tmp.tile([128, W - 1], DT.float32, tag="dmh")
                nc.vector.scalar_tensor_tensor(out=ddh, in0=dd, scalar=0.5, in1=mv[:, 1:],
                                               op0=OP.mult, op1=OP.mult)
                k32b = tmp.tile([128, W - 1], DT.int32, tag="k32")
                nc.vector.tensor_copy(k32b, ddh)
                kfb = tmp.tile([128, W - 1], DT.float32, tag="kf")
                nc.vector.tensor_copy(kfb, k32b)
                vw = tmp.tile([128, W - 1], DT.float32, tag="u")
                nc.vector.tensor_tensor(out=vw, in0=ddh, in1=kfb, op=OP.subtract)
                sqd = tmpb.tile([128, W - 1], DT.bfloat16, tag="sq")
                nc.scalar.activation(out=sqd, in_=vw, func=AF.Square, scale=2.0 * PI,
                                     accum_out=st[:, C_DSM:C_DSM + 1])

                # --- trajectory smoothness: |acc| (sqrt deferred) ---
                sqas = []
                for f in (0, 1):
                    vel = tmpb.tile([128, W - 1], DT.bfloat16, tag="vel")
                    nc.vector.tensor_tensor(out=vel, in0=rt[:, F + f::F],
                                            in1=rt[:, f:(W - 1) * F:F], op=OP.subtract)
                    acc = tmpb.tile([128, W - 2], DT.bfloat16, tag="acc")
                    nc.vector.tensor_tensor(out=acc, in0=vel[:, 1:], in1=vel[:, :W - 2],
                                            op=OP.subtract)
                    sqa = tmpb.tile([128, W - 2], DT.bfloat16, tag="sqa")
                    nc.vector.tensor_tensor(out=sqa, in0=acc, in1=acc, op=OP.mult)
                    sqas.append(sqa)
                msq = keep.tile([128, W - 2], DT.bfloat16, tag="msq")
                nc.vector.tensor_tensor(out=msq, in0=sqas[0], in1=sqas[1], op=OP.add)
                sts.append(st)
                mss.append(msq)
                lts.append(lt)

            # ---------------- sqrt phase (sqrt_and_others ACT set) ------------
            for b in range(NBLK):
                W = ws[b]
                amag = tmpb.tile([128, W - 2], DT.bfloat16, tag="amag")
                nc.scalar.activation(out=amag, in_=mss[b], func=AF.Sqrt, scale=1.0,
                                     bias=epsb[:, 0:1])
                mva = tmpb.tile([128, W], DT.bfloat16, tag="mva")
                nc.vector.tensor_scalar(out=mva, in0=itf[:, :W], scalar1=lts[b][:, 0:1],
                                        scalar2=None, op0=OP.is_lt)
                jk2 = tmpb.tile([128, W - 2], DT.bfloat16, tag="jk")
                nc.vector.scalar_tensor_tensor(out=jk2, in0=amag, scalar=1.0,
                                               in1=mva[:, 2:], op0=OP.mult, op1=OP.mult,
                                               accum_out=sts[b][:, C_TSM:C_TSM + 1])

            # ---------------- KL phase (exp_and_others ACT set) ---------------
            for b in range(NBLK):
                mut = tmp.tile([128, LATENT], DT.float32, tag="mut")
                lvt = tmp.tile([128, LATENT], DT.float32, tag="lvt")
                nc.sync.dma_start(out=mut, in_=muv[b])
                nc.sync.dma_start(out=lvt, in_=lvv[b])
                lvc = tmp.tile([128, LATENT], DT.float32, tag="lvc")
                nc.vector.tensor_scalar(out=lvc, in0=lvt, scalar1=10.0, scalar2=-10.0,
                                        op0=OP.min, op1=OP.max)
                elv = tmp.tile([128, LATENT], DT.float32, tag="elv")
                nc.scalar.activation(out=elv, in_=lvc, func=AF.Exp, scale=1.0,
                                     accum_out=sts[b][:, C_ELV:C_ELV + 1])
                jk3 = tmp.tile([128, LATENT], DT.float32, tag="jk3")
                nc.vector.scalar_tensor_tensor(out=jk3, in0=lvc, scalar=1.0, in1=elv,
                                               op0=OP.mult, op1=OP.subtract,
                                               accum_out=sts[b][:, C_LVE:C_LVE + 1])
                sq_mu = tmp.tile([128, LATENT], DT.float32, tag="jk3")
                nc.scalar.activation(out=sq_mu, in_=mut, func=AF.Square, scale=1.0,
                                     accum_out=sts[b][:, C_MU2:C_MU2 + 1])
                nc.sync.dma_start(out=outv[b], in_=sts[b])
    nc.compile()
    return nc


def _get_nc(ws):
    key = tuple(ws)
    if key not in _CACHE:
        _CACHE[key] = _build_nc(key)
    return _CACHE[key]


def _plan(lens_i):
    """Length-sorted, core-balanced sample permutation + per-block widths."""
    perm = np.argsort(-lens_i, kind="stable")
    slen = lens_i[perm]
    ws = []
    for b in range(NBLK):
        w = int(slen[b * 128 * NCORES])  # max length among this block's cohort
        w = max(w, 4)
        w += w & 1  # even width for DVE 2x modes
        w = min(w, L)
        ws.append(w)
    return perm, ws


def kernel(reconstruction, target, mu, logvar, predicted_length_ratio, seq_lengths):
    rec = np.asarray(reconstruction, dtype=np.float32).reshape(B, L * F)
    tgt = np.asarray(target, dtype=np.float32).reshape(B, L * F)
    mu_np = np.asarray(mu, dtype=np.float32)
    lv_np = np.asarray(logvar, dtype=np.float32)
    lens_i = np.asarray(seq_lengths).astype(np.int64)
    lens_f = lens_i.astype(np.float32).reshape(B, 1)

    perm, ws = _plan(lens_i)
    nc = _get_nc(ws)

    in_maps = []
    for c in range(NCORES):
        rows = perm[c::NCORES]  # 512 global sample indices, length-sorted desc
        m = {
            "mu": np.ascontiguousarray(mu_np[rows]),
            "lv": np.ascontiguousarray(lv_np[rows]),
            "lens": np.ascontiguousarray(lens_f[rows]),
        }
        for b in range(NBLK):
            br = rows[b * 128:(b + 1) * 128]
            m[f"r{b}"] = np.ascontiguousarray(rec[br, :ws[b] * F])
            m[f"t{b}"] = np.ascontiguousarray(tgt[br, :ws[b] * F])
        in_maps.append(m)

    res = bass_utils.run_bass_kernel_spmd(nc, in_maps, core_ids=list(range(NCORES)))

    # un-permute partials back to original sample order
    parts_p = np.concatenate([res.results[c]["out"] for c in range(NCORES)], axis=0)
    parts = np.empty_like(parts_p, dtype=np.float64)
    order = np.empty(B, dtype=np.int64)
    for c in range(NCORES):
        order[c * SPC:(c + 1) * SPC] = perm[c::NCORES]
    parts[order] = parts_p.astype(np.float64)

    # ---------------- host-side O(B) finishing math ----------------
    lens = lens_i.astype(np.float64)
    msum = parts[:, C_MSUM].sum()
    eps = 1e-8

    position_loss = (parts[:, C_Q0].sum() + parts[:, C_Q1].sum()) / (2.0 * msum + eps)
    time_loss = parts[:, C_Q2].sum() / (msum + eps)
    speed_loss = parts[:, C_Q4].sum() / (msum + eps)
    direction_loss = 2.0 * parts[:, C_DIR].sum() / (msum + eps)

    # endpoint loss (host gather, O(B))
    last = np.clip(lens_i - 1, 0, None)
    r3 = rec.reshape(B, L, F)
    t3 = tgt.reshape(B, L, F)
    ar = np.arange(B)
    ep_pred = r3[ar, last, 0:2].astype(np.float64)
    ep_true = t3[ar, last, 0:2].astype(np.float64)
    ep_mse = ((ep_pred - ep_true) ** 2).mean(axis=1)
    endpoint_loss = np.where(lens_i > 0, ep_mse, 0.0).sum() / B

    # length ratio loss (host, O(B))
    plr = np.asarray(predicted_length_ratio, dtype=np.float64).reshape(B)
    true_ratio = lens / L
    length_loss = ((true_ratio - plr) ** 2).sum() / B

    dcount = np.maximum(lens - 1.0, 1.0)
    acount = np.maximum(lens - 2.0, 1.0)
    gt2 = lens_i > 2

    # speed deceleration
    decel = parts[:, C_DECEL] / dcount
    s0 = r3[:, 0, 4].astype(np.float64)
    send = r3[ar, last, 4].astype(np.float64)
    start_pen = np.maximum(0.3 - s0, 0.0)
    end_pen = np.maximum(send - 0.2, 0.0)
    speed_decel_loss = np.where(gt2, decel + 0.5 * (start_pen + end_pen), 0.0).sum() / B

    dir_smooth_loss = np.where(gt2, parts[:, C_DSM] / dcount, 0.0).sum() / B
    traj_smooth_loss = np.where(gt2, parts[:, C_TSM] / acount, 0.0).sum() / B

    # C_LVE holds sum(clip(logvar) - exp(clip(logvar)))
    kl_per = -0.5 * (LATENT + parts[:, C_LVE] - parts[:, C_MU2])
    kl_loss = kl_per.mean()

    reconstruction_loss = (W_POS * position_loss + W_TIME * time_loss
                           + W_DIR * direction_loss + W_EP * endpoint_loss
                           + W_LEN * length_loss + W_SPD * speed_loss
                           + W_DECEL * speed_decel_loss + W_DSM * dir_smooth_loss
                           + W_TSM * traj_smooth_loss)
    total = reconstruction_loss + W_KL * kl_loss
    return np.float32(total)


# revision 8
# speedup vs baseline: 1.4586x; 1.4586x over previous
"""CVAE loss kernel for Trainium2 (8 NeuronCores, data-parallel over batch).

Strategy:
  - Host: sort samples by sequence length, deal round-robin to the 8 cores
    (identical length profile per core), and lay each 128-sample block out
    feature-major in bf16 with the invalid tail zeroed. Each block only
    carries its own max length Wb of the 1024 timesteps (~62% of dense).
  - Device: samples on SBUF partitions; every elementwise op is unit-stride
    bf16 (DVE 2x mode). Because tails are zero, masked reductions collapse
    to plain fused accumulations (activation/tensor_scalar accum_out); the
    single boundary column of the difference terms is corrected exactly on
    the host. Trig is range-reduced with an int-round trick (Sin on the
    scalar engine only accepts [-pi, pi]).
  - Host: O(B) finishing math (boundary corrections, endpoint gathers,
    per-sample normalization, final weighted sum).
"""

import numpy as np
import ml_dtypes

import concourse.bacc as bacc
import concourse.tile as tile
from concourse import mybir, bass_utils

# Problem constants (hardcoded per contest rules).
B, L, F = 4096, 1024, 5
LATENT = 128
NCORES = 8
SPC = B // NCORES          # samples per core = 512
NBLK = SPC // 128          # partition blocks per core = 4
PI = float(np.pi)

# loss weights (match CVAELoss defaults)
W_POS, W_TIME, W_DIR, W_EP, W_LEN = 3.0, 0.5, 3.0, 10.0, 2.0
W_SPD, W_DECEL, W_DSM, W_TSM, W_KL = 1.5, 2.0, 2.5, 3.0, 0.01

OP = mybir.AluOpType
AF = mybir.ActivationFunctionType
DT = mybir.dt
BF16 = ml_dtypes.bfloat16

# partials columns
C_MSUM, C_Q0, C_Q1, C_Q2, C_Q4, C_DIR, C_DECEL, C_DSM, C_TSM, C_LVE, C_MU2 = range(11)
NCOL = 12  # padded to 12 for alignment

_CACHE = {}


def _wrap_chain(nc, tmp, tmpb, src, width, tag_prefix):
    """u2 such that pi*u2 == wrap(2*pi*(src/2)) == src wrapped to [-pi,pi] scale.

    k = rint(src/2); u2 = src - 2k  (in [-1, 1]).  All bf16, DVE 2x.
    """
    k32 = tmp.tile([128, width], DT.int32, tag=f"{tag_prefix}k")
    nc.vector.tensor_scalar(out=k32, in0=src, scalar1=0.5, scalar2=None, op0=OP.mult)
    kf2 = tmpb.tile([128, width], DT.bfloat16, tag=f"{tag_prefix}kf")
    nc.vector.tensor_scalar(out=kf2, in0=k32, scalar1=2.0, scalar2=None, op0=OP.mult)
    u2 = tmpb.tile([128, width], DT.bfloat16, tag=f"{tag_prefix}u")
    nc.vector.tensor_tensor(out=u2, in0=src, in1=kf2, op=OP.subtract)
    return u2


def _build_nc(ws):
    """ws: per-block free-dim widths (max sequence length in each block)."""
    nc = bacc.Bacc("TRN2", target_bir_lowering=False, debug=False)
    rts = [(nc.dram_tensor(f"r{b}", [128, F * ws[b]], DT.bfloat16, kind="ExternalInput"),
            nc.dram_tensor(f"t{b}", [128, F * ws[b]], DT.bfloat16, kind="ExternalInput"))
           for b in range(NBLK)]
    mu = nc.dram_tensor("mu", [SPC, LATENT], DT.float32, kind="ExternalInput")
    lv = nc.dram_tensor("lv", [SPC, LATENT], DT.float32, kind="ExternalInput")
    lens = nc.dram_tensor("lens", [SPC, 1], DT.float32, kind="ExternalInput")
    out = nc.dram_tensor("out", [SPC, NCOL], DT.float32, kind="ExternalOutput")

    muv = mu.ap().rearrange("(b p) d -> b p d", p=128)
    lvv = lv.ap().rearrange("(b p) d -> b p d", p=128)
    lensv = lens.ap().rearrange("(b p) d -> b p d", p=128)
    outv = out.ap().rearrange("(b p) d -> b p d", p=128)

    with tile.TileContext(nc) as tc:
        with (
            tc.tile_pool(name="io", bufs=2) as io,          # big R/T tiles
            tc.tile_pool(name="tmp", bufs=2) as tmp,        # int/f32 scratch
            tc.tile_pool(name="tmpb", bufs=2) as tmpb,      # bf16 scratch
            tc.tile_pool(name="keep", bufs=NBLK) as keep,   # persists across phases
            tc.tile_pool(name="one", bufs=1) as one,        # constants
        ):
            # constants
            it32 = one.tile([128, L], DT.int32)
            nc.gpsimd.iota(it32, pattern=[[1, L]], base=0, channel_multiplier=0)
            itf = one.tile([128, L], DT.float32)
            nc.vector.tensor_copy(itf, it32)
            epsb = one.tile([128, 1], DT.float32)
            nc.vector.memset(epsb, 1e-8)

            sts, mss, lts = [], [], []
            # ---------------- main block loop (trig_and_small ACT set) --------
            for b in range(NBLK):
                W = ws[b]
                rt = io.tile([128, F * W], DT.bfloat16, tag="rt")
                tt = io.tile([128, F * W], DT.bfloat16, tag="tt")
                nc.sync.dma_start(out=rt, in_=rts[b][0].ap())
                nc.sync.dma_start(out=tt, in_=rts[b][1].ap())
                lt = keep.tile([128, 1], DT.float32, tag="lt")
                nc.sync.dma_start(out=lt, in_=lensv[b])
                st = keep.tile([128, NCOL], DT.float32, tag="st")

                def rf(f):
                    return rt[:, f * W:(f + 1) * W]

                def tf(f):
                    return tt[:, f * W:(f + 1) * W]

                # --- squared diffs for features 0,1,2,4 (tails are zero) ---
                for f, col in ((0, C_Q0), (1, C_Q1), (2, C_Q2), (4, C_Q4)):
                    d = tmpb.tile([128, W], DT.bfloat16, tag="d")
                    nc.vector.tensor_tensor(out=d, in0=rf(f), in1=tf(f), op=OP.subtract)
                    sq = tmpb.tile([128, W], DT.bfloat16, tag="sq")
                    nc.scalar.activation(out=sq, in_=d, func=AF.Square, scale=1.0,
                                         accum_out=st[:, col:col + 1])

                # --- direction loss: sum(1-cos(pi*d3)) = 2*sum(sin^2(pi*d3/2)) ---
                d3 = tmpb.tile([128, W], DT.bfloat16, tag="d")
                nc.vector.tensor_tensor(out=d3, in0=rf(3), in1=tf(3), op=OP.subtract)
                u2 = _wrap_chain(nc, tmp, tmpb, d3, W, "dir")
                s = tmpb.tile([128, W], DT.bfloat16, tag="s")
                nc.scalar.activation(out=s, in_=u2, func=AF.Sin, scale=PI / 2.0)
                sq3 = tmpb.tile([128, W], DT.bfloat16, tag="sq")
                nc.scalar.activation(out=sq3, in_=s, func=AF.Square, scale=1.0,
                                     accum_out=st[:, C_DIR:C_DIR + 1])

                # --- speed deceleration: relu of s-diff; boundary col fixed on host
                sdiff = tmpb.tile([128, W - 1], DT.bfloat16, tag="sdiff")
                nc.vector.tensor_tensor(out=sdiff, in0=rf(4)[:, 1:], in1=rf(4)[:, :W - 1],
                                        op=OP.subtract)
                jk1 = tmpb.tile([128, W - 1], DT.bfloat16, tag="jk")
                nc.vector.tensor_scalar(out=jk1, in0=sdiff, scalar1=0.0, scalar2=None,
                                        op0=OP.max, op1=OP.add,
                                        accum_out=st[:, C_DECEL:C_DECEL + 1])

                # --- direction smoothness: wrap(pi*ddiff)^2; boundary on host ---
                dd = tmpb.tile([128, W - 1], DT.bfloat16, tag="sdiff")
                nc.vector.tensor_tensor(out=dd, in0=rf(3)[:, 1:], in1=rf(3)[:, :W - 1],
                                        op=OP.subtract)
                ud = _wrap_chain(nc, tmp, tmpb, dd, W - 1, "dsm")
                sqd = tmpb.tile([128, W - 1], DT.bfloat16, tag="jk")
                nc.scalar.activation(out=sqd, in_=ud, func=AF.Square, scale=PI,
                                     accum_out=st[:, C_DSM:C_DSM + 1])

                # --- trajectory smoothness: |acc| (sqrt + mask deferred) ---
                sqas = []
                for f in (0, 1):
                    vel = tmpb.tile([128, W - 1], DT.bfloat16, tag="vel")
                    nc.vector.tensor_tensor(out=vel, in0=rf(f)[:, 1:], in1=rf(f)[:, :W - 1],
                                            op=OP.subtract)
                    acc = tmpb.tile([128, W - 2], DT.bfloat16, tag="acc")
                    nc.vector.tensor_tensor(out=acc, in0=vel[:, 1:], in1=vel[:, :W - 2],
                                            op=OP.subtract)
                    sqa = tmpb.tile([128, W - 2], DT.bfloat16, tag="sqa")
                    nc.gpsimd.tensor_tensor(out=sqa, in0=acc, in1=acc, op=OP.mult)
                    sqas.append(sqa)
                msq = keep.tile([128, W - 2], DT.bfloat16, tag="msq")
                nc.vector.tensor_tensor(out=msq, in0=sqas[0], in1=sqas[1], op=OP.add)
                sts.append(st)
                mss.append(msq)
                lts.append(lt)

            tc.no_sync_barrier()
            # ---------------- sqrt phase (sqrt_and_others ACT set) ------------
            for b in range(NBLK):
                W = ws[b]
                amag = tmpb.tile([128, W - 2], DT.bfloat16, tag="amag")
                nc.scalar.activation(out=amag, in_=mss[b], func=AF.Sqrt, scale=1.0,
                                     bias=epsb[:, 0:1])
                mva = tmpb.tile([128, W], DT.bfloat16, tag="mva")
                nc.vector.tensor_scalar(out=mva, in0=itf[:, :W], scalar1=lts[b][:, 0:1],
                                        scalar2=None, op0=OP.is_lt, op1=OP.add,
                                        accum_out=sts[b][:, C_MSUM:C_MSUM + 1])
                jk2 = tmpb.tile([128, W - 2], DT.bfloat16, tag="jk")
                nc.vector.scalar_tensor_tensor(out=jk2, in0=amag, scalar=1.0,
                                               in1=mva[:, 2:], op0=OP.mult, op1=OP.mult,
                                               accum_out=sts[b][:, C_TSM:C_TSM + 1])

            tc.no_sync_barrier()
            # ---------------- KL phase (exp_and_others ACT set) ---------------
            for b in range(NBLK):
                mut = tmp.tile([128, LATENT], DT.float32, tag="mut")
                lvt = tmp.tile([128, LATENT], DT.float32, tag="lvt")
                nc.sync.dma_start(out=mut, in_=muv[b])
                nc.sync.dma_start(out=lvt, in_=lvv[b])
                lvc = tmp.tile([128, LATENT], DT.float32, tag="lvc")
                nc.vector.tensor_scalar(out=lvc, in0=lvt, scalar1=10.0, scalar2=-10.0,
                                        op0=OP.min, op1=OP.max)
                elv = tmp.tile([128, LATENT], DT.float32, tag="elv")
                nc.scalar.activation(out=elv, in_=lvc, func=AF.Exp, scale=1.0)
                jk3 = tmp.tile([128, LATENT], DT.float32, tag="jk3")
                nc.vector.scalar_tensor_tensor(out=jk3, in0=lvc, scalar=1.0, in1=elv,
                                               op0=OP.mult, op1=OP.subtract,
                                               accum_out=sts[b][:, C_LVE:C_LVE + 1])
                sq_mu = tmp.tile([128, LATENT], DT.float32, tag="jk3")
                nc.scalar.activation(out=sq_mu, in_=mut, func=AF.Square, scale=1.0,
                                     accum_out=sts[b][:, C_MU2:C_MU2 + 1])
                nc.sync.dma_start(out=outv[b], in_=sts[b])
    nc.compile()
    return nc


def _get_nc(ws):
    key = tuple(ws)
    if key not in _CACHE:
        _CACHE[key] = _build_nc(key)
    return _CACHE[key]


def _plan(lens_i):
    """Length-sorted, core-balanced sample permutation + per-block widths."""
    perm = np.argsort(-lens_i, kind="stable")
    slen = lens_i[perm]
    ws = []
    for b in range(NBLK):
        w = int(slen[b * 128 * NCORES])  # max length among this block's cohort
        w = max(w, 4)
        w += w & 1  # even width for DVE 2x modes
        w = min(w, L)
        ws.append(w)
    return perm, ws


def _np_wrap(x):
    return np.arctan2(np.sin(x), np.cos(x))


def kernel(reconstruction, target, mu, logvar, predicted_length_ratio, seq_lengths):
    rec = np.asarray(reconstruction, dtype=np.float32).reshape(B, L, F)
    tgt = np.asarray(target, dtype=np.float32).reshape(B, L, F)
    mu_np = np.asarray(mu, dtype=np.float32)
    lv_np = np.asarray(logvar, dtype=np.float32)
    lens_i = np.asarray(seq_lengths).astype(np.int64)
    lens_f = lens_i.astype(np.float32).reshape(B, 1)

    perm, ws = _plan(lens_i)
    nc = _get_nc(ws)

    cols = np.arange(L)
    in_maps = []
    for c in range(NCORES):
        rows = perm[c::NCORES]  # 512 global sample indices, length-sorted desc
        m = {
            "mu": np.ascontiguousarray(mu_np[rows]),
            "lv": np.ascontiguousarray(lv_np[rows]),
            "lens": np.ascontiguousarray(lens_f[rows]),
        }
        for b in range(NBLK):
            br = rows[b * 128:(b + 1) * 128]
            wb = ws[b]
            invalid = cols[None, :wb, None] >= lens_i[br][:, None, None]  # (128,wb,1)
            for name, src in ((f"r{b}", rec), (f"t{b}", tgt)):
                x = src[br][:, :wb, :].copy()           # (128, wb, F)
                np.copyto(x, 0.0, where=invalid)
                m[name] = np.ascontiguousarray(
                    x.transpose(0, 2, 1)).reshape(128, F * wb).astype(BF16)
        in_maps.append(m)

    res = bass_utils.run_bass_kernel_spmd(nc, in_maps, core_ids=list(range(NCORES)))

    # un-permute partials back to original sample order
    parts_p = np.concatenate([res.results[c]["out"] for c in range(NCORES)], axis=0)
    parts = np.empty_like(parts_p, dtype=np.float64)
    order = np.empty(B, dtype=np.int64)
    for c in range(NCORES):
        order[c * SPC:(c + 1) * SPC] = perm[c::NCORES]
    parts[order] = parts_p.astype(np.float64)

    # per-sample block width (for boundary-junk corrections)
    rank = np.empty(B, dtype=np.int64)
    rank[perm] = np.arange(B)
    wb_s = np.asarray(ws, dtype=np.int64)[rank // (128 * NCORES)]

    # ---------------- host-side O(B) finishing math ----------------
    lens = lens_i.astype(np.float64)
    msum = lens.sum()
    eps = 1e-8
    ar = np.arange(B)
    last = np.clip(lens_i - 1, 0, None)

    position_loss = (parts[:, C_Q0].sum() + parts[:, C_Q1].sum()) / (2.0 * msum + eps)
    time_loss = parts[:, C_Q2].sum() / (msum + eps)
    speed_loss = parts[:, C_Q4].sum() / (msum + eps)
    direction_loss = 2.0 * parts[:, C_DIR].sum() / (msum + eps)

    # endpoint loss (host gather, O(B))
    ep_mse = ((rec[ar, last, 0:2].astype(np.float64)
               - tgt[ar, last, 0:2].astype(np.float64)) ** 2).mean(axis=1)
    endpoint_loss = np.where(lens_i > 0, ep_mse, 0.0).sum() / B

    # length ratio loss (host, O(B))
    plr = np.asarray(predicted_length_ratio, dtype=np.float64).reshape(B)
    true_ratio = lens / L
    length_loss = ((true_ratio - plr) ** 2).sum() / B

    dcount = np.maximum(lens - 1.0, 1.0)
    acount = np.maximum(lens - 2.0, 1.0)
    gt2 = lens_i > 2

    # boundary-junk corrections: the device sums include one junk column at
    # l = len-1 (value diff against the zeroed tail) whenever len <= Wb-1.
    has_junk = (lens_i >= 1) & (lens_i <= wb_s - 1)
    s_last = rec[ar, last, 4].astype(np.float64)
    bf = lambda v: v.astype(BF16).astype(np.float64)
    s_last_b = bf(rec[ar, last, 4])
    d3_last_b = bf(rec[ar, last, 3])
    decel_junk = np.where(has_junk, np.maximum(-s_last_b, 0.0), 0.0)
    # emulate the device's wrap chain (incl. bf16 rounding of u2) exactly
    dd_j = -d3_last_b
    u2_j = bf(dd_j - 2.0 * np.round(0.5 * dd_j))
    dsm_junk = np.where(has_junk, (PI * u2_j) ** 2, 0.0)

    decel = (parts[:, C_DECEL] - decel_junk) / dcount
    s0 = rec[:, 0, 4].astype(np.float64)
    start_pen = np.maximum(0.3 - s0, 0.0)
    end_pen = np.maximum(s_last - 0.2, 0.0)
    speed_decel_loss = np.where(gt2, decel + 0.5 * (start_pen + end_pen), 0.0).sum() / B

    dir_smooth_loss = np.where(gt2, (parts[:, C_DSM] - dsm_junk) / dcount, 0.0).sum() / B
    traj_smooth_loss = np.where(gt2, parts[:, C_TSM] / acount, 0.0).sum() / B

    # C_LVE holds sum(clip(logvar) - exp(clip(logvar)))
    kl_per = -0.5 * (LATENT + parts[:, C_LVE] - parts[:, C_MU2])
    kl_loss = kl_per.mean()

    reconstruction_loss = (W_POS * position_loss + W_TIME * time_loss
                           + W_DIR * direction_loss + W_EP * endpoint_loss
                           + W_LEN * length_loss + W_SPD * speed_loss
                           + W_DECEL * speed_decel_loss + W_DSM * dir_smooth_loss
                           + W_TSM * traj_smooth_loss)
    total = reconstruction_loss + W_KL * kl_loss
    return np.float32(total)


# revision 13
# speedup vs baseline: 1.5110x; 1.0360x over previous
"""CVAE loss kernel for Trainium2 (8 NeuronCores, data-parallel over batch).

Strategy:
  - Host: sort samples by sequence length, deal round-robin to the 8 cores
    (identical length profile per core), and lay each 128-sample block out
    feature-major in bf16 with the invalid tail zeroed. Each block only
    carries its own max length Wb of the 1024 timesteps (~62% of dense).
    Feature 2 is pre-scaled by sqrt(W_TIME/W_SPD) so all four squared-diff
    terms share one weight and reduce in a single fused accumulation.
  - Device: samples on SBUF partitions; every elementwise op is unit-stride
    bf16 (DVE 2x mode). Because tails are zero, masked reductions collapse
    to plain fused accumulations (activation/tensor_scalar accum_out); the
    boundary columns of the difference terms are corrected exactly on the
    host. Trig is range-reduced with an int-round trick (Sin on the scalar
    engine only accepts [-pi, pi]).
  - Host: O(B) finishing math (boundary corrections, endpoint gathers,
    per-sample normalization, final weighted sum).
"""

import numpy as np
import ml_dtypes

import concourse.bacc as bacc
import concourse.tile as tile
from concourse import mybir, bass_utils

# Problem constants (hardcoded per contest rules).
B, L, F = 4096, 1024, 5
LATENT = 128
NCORES = 8
SPC = B // NCORES          # samples per core = 512
NBLK = SPC // 128          # partition blocks per core = 4
PI = float(np.pi)

# loss weights (match CVAELoss defaults)
W_POS, W_TIME, W_DIR, W_EP, W_LEN = 3.0, 0.5, 3.0, 10.0, 2.0
W_SPD, W_DECEL, W_DSM, W_TSM, W_KL = 1.5, 2.0, 2.5, 3.0, 0.01

G2 = float(np.sqrt(W_TIME / W_SPD))  # feature-2 prescale so sq-terms merge

OP = mybir.AluOpType
AF = mybir.ActivationFunctionType
DT = mybir.dt
BF16 = ml_dtypes.bfloat16

# partials columns (per-sample)
C_Q, C_DIR, C_DECEL, C_DSM, C_TSM = range(5)
NCOL = 6  # padded

_CACHE = {}


def _wrap_chain(nc, tmp, tmpb, src, width, tag_prefix):
    """u2 such that pi*u2 == wrap(pi*src) to [-pi,pi].

    k = rint(src/2); u2 = src - 2k  (in [-1, 1]).  All bf16, DVE 2x.
    """
    k32 = tmp.tile([128, width], DT.int32, tag=f"{tag_prefix}k")
    nc.vector.tensor_scalar(out=k32, in0=src, scalar1=0.5, scalar2=None, op0=OP.mult)
    kf2 = tmpb.tile([128, width], DT.bfloat16, tag=f"{tag_prefix}kf")
    nc.vector.tensor_scalar(out=kf2, in0=k32, scalar1=2.0, scalar2=None, op0=OP.mult)
    u2 = tmpb.tile([128, width], DT.bfloat16, tag=f"{tag_prefix}u")
    nc.vector.tensor_tensor(out=u2, in0=src, in1=kf2, op=OP.subtract)
    return u2


def _build_nc(ws):
    """ws: per-block free-dim widths (max sequence length in each block)."""
    nc = bacc.Bacc("TRN2", target_bir_lowering=False, debug=False)
    rts = [(nc.dram_tensor(f"r{b}", [128, F * ws[b]], DT.bfloat16, kind="ExternalInput"),
            nc.dram_tensor(f"t{b}", [128, F * ws[b]], DT.bfloat16, kind="ExternalInput"))
           for b in range(NBLK)]
    mu = nc.dram_tensor("mu", [SPC, LATENT], DT.float32, kind="ExternalInput")
    lv = nc.dram_tensor("lv", [SPC, LATENT], DT.float32, kind="ExternalInput")
    out = nc.dram_tensor("out", [SPC, NCOL], DT.float32, kind="ExternalOutput")
    klout = nc.dram_tensor("klout", [128, 2], DT.float32, kind="ExternalOutput")

    # KL data viewed so each partition carries NBLK samples' latent rows
    muv = mu.ap().rearrange("(b p) d -> p b d", p=128)
    lvv = lv.ap().rearrange("(b p) d -> p b d", p=128)
    outv = out.ap().rearrange("(b p) d -> b p d", p=128)

    with tile.TileContext(nc) as tc:
        with (
            tc.tile_pool(name="io", bufs=2) as io,          # big R/T tiles
            tc.tile_pool(name="tmp", bufs=2) as tmp,        # int/f32 scratch
            tc.tile_pool(name="tmpb", bufs=2) as tmpb,      # bf16 scratch
            tc.tile_pool(name="psj", bufs=1, space="PSUM") as psj,  # junk sinks
            tc.tile_pool(name="keep", bufs=NBLK) as keep,   # persists across phases
        ):
            sts, mss = [], []
            # ---------------- main block loop (trig_and_small ACT set) --------
            for b in range(NBLK):
                W = ws[b]
                rt = io.tile([128, F * W], DT.bfloat16, tag="rt")
                tt = io.tile([128, F * W], DT.bfloat16, tag="tt")
                h = F * W // 2
                nc.sync.dma_start(out=rt[:, :h], in_=rts[b][0].ap()[:, :h])
                nc.scalar.dma_start(out=rt[:, h:], in_=rts[b][0].ap()[:, h:])
                nc.sync.dma_start(out=tt[:, :h], in_=rts[b][1].ap()[:, :h])
                nc.scalar.dma_start(out=tt[:, h:], in_=rts[b][1].ap()[:, h:])
                st = keep.tile([128, NCOL], DT.float32, tag="st")

                def rf(f):
                    return rt[:, f * W:(f + 1) * W]

                def tf(f):
                    return tt[:, f * W:(f + 1) * W]

                # --- squared diffs for features 0,1,2,4, one fused accum ---
                dq = tmpb.tile([128, 4 * W], DT.bfloat16, tag="dq")
                for i, f in enumerate((0, 1, 2, 4)):
                    eng = nc.gpsimd if f in (2, 4) else nc.vector
                    eng.tensor_tensor(out=dq[:, i * W:(i + 1) * W], in0=rf(f),
                                      in1=tf(f), op=OP.subtract)
                sqj = psj.tile([128, 4 * ws[0]], DT.float32, tag="sqj")
                nc.scalar.activation(out=sqj[:, :4 * W], in_=dq, func=AF.Square,
                                     scale=1.0, accum_out=st[:, C_Q:C_Q + 1])

                # --- direction loss: sum(1-cos(pi*d3)) = 2*sum(sin^2(pi*d3/2)) ---
                d3 = tmpb.tile([128, W], DT.bfloat16, tag="d")
                nc.vector.tensor_tensor(out=d3, in0=rf(3), in1=tf(3), op=OP.subtract)
                u2 = _wrap_chain(nc, tmp, tmpb, d3, W, "dir")
                s = tmpb.tile([128, W], DT.bfloat16, tag="s")
                nc.scalar.activation(out=s, in_=u2, func=AF.Sin, scale=PI / 2.0)
                sq3 = tmpb.tile([128, W], DT.bfloat16, tag="jk")
                nc.scalar.activation(out=sq3, in_=s, func=AF.Square, scale=1.0,
                                     accum_out=st[:, C_DIR:C_DIR + 1])

                # --- speed deceleration: relu of s-diff; boundary col fixed on host
                sdiff = tmpb.tile([128, W - 1], DT.bfloat16, tag="sdiff")
                nc.vector.tensor_tensor(out=sdiff, in0=rf(4)[:, 1:], in1=rf(4)[:, :W - 1],
                                        op=OP.subtract)
                jk1 = tmpb.tile([128, W - 1], DT.bfloat16, tag="jk")
                nc.vector.tensor_scalar(out=jk1, in0=sdiff, scalar1=0.0, scalar2=None,
                                        op0=OP.max, op1=OP.add,
                                        accum_out=st[:, C_DECEL:C_DECEL + 1])

                # --- direction smoothness: wrap(pi*ddiff)^2; boundary on host ---
                dd = tmpb.tile([128, W - 1], DT.bfloat16, tag="sdiff")
                nc.vector.tensor_tensor(out=dd, in0=rf(3)[:, 1:], in1=rf(3)[:, :W - 1],
                                        op=OP.subtract)
                ud = _wrap_chain(nc, tmp, tmpb, dd, W - 1, "dsm")
                sqd = tmpb.tile([128, W - 1], DT.bfloat16, tag="jk")
                nc.scalar.activation(out=sqd, in_=ud, func=AF.Square, scale=PI,
                                     accum_out=st[:, C_DSM:C_DSM + 1])

                # --- trajectory smoothness: |acc|; boundary cols fixed on host ---
                sqas = []
                for f in (0, 1):
                    vel = tmpb.tile([128, W - 1], DT.bfloat16, tag="vel")
                    nc.vector.tensor_tensor(out=vel, in0=rf(f)[:, 1:], in1=rf(f)[:, :W - 1],
                                            op=OP.subtract)
                    acc = tmpb.tile([128, W - 2], DT.bfloat16, tag="acc")
                    nc.vector.tensor_tensor(out=acc, in0=vel[:, 1:], in1=vel[:, :W - 2],
                                            op=OP.subtract)
                    sqa = tmpb.tile([128, W - 2], DT.bfloat16, tag="sqa")
                    nc.gpsimd.tensor_tensor(out=sqa, in0=acc, in1=acc, op=OP.mult)
                    sqas.append(sqa)
                msq = keep.tile([128, W - 2], DT.bfloat16, tag="msq")
                nc.vector.tensor_tensor(out=msq, in0=sqas[0], in1=sqas[1], op=OP.add)
                sts.append(st)
                mss.append(msq)

            tc.no_sync_barrier()
            # ---------------- sqrt phase (sqrt_and_others ACT set) ------------
            for b in range(NBLK):
                W = ws[b]
                amj = tmpb.tile([128, W - 2], DT.bfloat16, tag="jk")
                nc.scalar.activation(out=amj, in_=mss[b], func=AF.Sqrt,
                                     scale=1.0, accum_out=sts[b][:, C_TSM:C_TSM + 1])
                nc.sync.dma_start(out=outv[b], in_=sts[b])

            tc.no_sync_barrier()
            # ---------------- KL phase (exp_and_others ACT set), one shot -----
            klt = keep.tile([128, 2], DT.float32, tag="klt")
            mut = tmp.tile([128, NBLK * LATENT], DT.float32, tag="mut")
            lvt = tmp.tile([128, NBLK * LATENT], DT.float32, tag="lvt")
            nc.sync.dma_start(
                out=mut[:, :].rearrange("p (b d) -> p b d", b=NBLK), in_=muv)
            nc.sync.dma_start(
                out=lvt[:, :].rearrange("p (b d) -> p b d", b=NBLK), in_=lvv)
            lvc = tmp.tile([128, NBLK * LATENT], DT.float32, tag="lvc")
            nc.vector.tensor_scalar(out=lvc, in0=lvt, scalar1=10.0, scalar2=-10.0,
                                    op0=OP.min, op1=OP.max)
            elv = tmp.tile([128, NBLK * LATENT], DT.float32, tag="elv")
            nc.scalar.activation(out=elv, in_=lvc, func=AF.Exp, scale=1.0)
            jk3 = tmp.tile([128, NBLK * LATENT], DT.float32, tag="jk3")
            nc.vector.scalar_tensor_tensor(out=jk3, in0=lvc, scalar=1.0, in1=elv,
                                           op0=OP.mult, op1=OP.subtract,
                                           accum_out=klt[:, 0:1])
            sq_mu = tmp.tile([128, NBLK * LATENT], DT.float32, tag="jk3")
            nc.scalar.activation(out=sq_mu, in_=mut, func=AF.Square, scale=1.0,
                                 accum_out=klt[:, 1:2])
            nc.sync.dma_start(out=klout.ap(), in_=klt)
    nc.compile()
    return nc


def _get_nc(ws):
    key = tuple(ws)
    if key not in _CACHE:
        _CACHE[key] = _build_nc(key)
    return _CACHE[key]


def _plan(lens_i):
    """Length-sorted, core-balanced sample permutation + per-block widths."""
    perm = np.argsort(-lens_i, kind="stable")
    slen = lens_i[perm]
    ws = []
    for b in range(NBLK):
        w = int(slen[b * 128 * NCORES])  # max length among this block's cohort
        w = max(w, 4)
        w += w & 1  # even width for DVE 2x modes
        w = min(w, L)
        ws.append(w)
    return perm, ws


def kernel(reconstruction, target, mu, logvar, predicted_length_ratio, seq_lengths):
    rec = np.asarray(reconstruction, dtype=np.float32).reshape(B, L, F)
    tgt = np.asarray(target, dtype=np.float32).reshape(B, L, F)
    mu_np = np.asarray(mu, dtype=np.float32)
    lv_np = np.asarray(logvar, dtype=np.float32)
    lens_i = np.asarray(seq_lengths).astype(np.int64)

    perm, ws = _plan(lens_i)
    nc = _get_nc(ws)

    gscale = np.asarray([1.0, 1.0, G2, 1.0, 1.0], dtype=np.float32)
    cols = np.arange(L)
    in_maps = []
    for c in range(NCORES):
        rows = perm[c::NCORES]  # 512 global sample indices, length-sorted desc
        m = {
            "mu": np.ascontiguousarray(mu_np[rows]),
            "lv": np.ascontiguousarray(lv_np[rows]),
        }
        for b in range(NBLK):
            br = rows[b * 128:(b + 1) * 128]
            wb = ws[b]
            invalid = cols[None, :wb, None] >= lens_i[br][:, None, None]  # (128,wb,1)
            for name, src in ((f"r{b}", rec), (f"t{b}", tgt)):
                x = src[br][:, :wb, :] * gscale                 # (128, wb, F)
                np.copyto(x, 0.0, where=invalid)
                m[name] = np.ascontiguousarray(
                    x.transpose(0, 2, 1)).reshape(128, F * wb).astype(BF16)
        in_maps.append(m)

    res = bass_utils.run_bass_kernel_spmd(nc, in_maps, core_ids=list(range(NCORES)))

    # un-permute partials back to original sample order
    parts_p = np.concatenate([res.results[c]["out"] for c in range(NCORES)], axis=0)
    parts = np.empty_like(parts_p, dtype=np.float64)
    order = np.empty(B, dtype=np.int64)
    for c in range(NCORES):
        order[c * SPC:(c + 1) * SPC] = perm[c::NCORES]
    parts[order] = parts_p.astype(np.float64)
    kl_sums = sum(np.asarray(res.results[c]["klout"], dtype=np.float64).sum(axis=0)
                  for c in range(NCORES))

    # per-sample block width (for boundary-junk corrections)
    rank = np.empty(B, dtype=np.int64)
    rank[perm] = np.arange(B)
    wb_s = np.asarray(ws, dtype=np.int64)[rank // (128 * NCORES)]

    # ---------------- host-side O(B) finishing math ----------------
    bf = lambda v: np.asarray(v, dtype=np.float32).astype(BF16).astype(np.float64)
    lens = lens_i.astype(np.float64)
    msum = lens.sum()
    eps = 1e-8
    ar = np.arange(B)
    last = np.clip(lens_i - 1, 0, None)

    # combined squared-diff term:
    # W_SPD*(A_q)/(msum+eps) == W_POS*pos + W_TIME*time + W_SPD*speed  (eps-exact
    # up to ~1e-15 relative, since position's denominator is 2*msum+eps)
    sq_term = W_SPD * parts[:, C_Q].sum() / (msum + eps)
    direction_loss = 2.0 * parts[:, C_DIR].sum() / (msum + eps)

    # endpoint loss (host gather, O(B))
    ep_mse = ((rec[ar, last, 0:2].astype(np.float64)
               - tgt[ar, last, 0:2].astype(np.float64)) ** 2).mean(axis=1)
    endpoint_loss = np.where(lens_i > 0, ep_mse, 0.0).sum() / B

    # length ratio loss (host, O(B))
    plr = np.asarray(predicted_length_ratio, dtype=np.float64).reshape(B)
    true_ratio = lens / L
    length_loss = ((true_ratio - plr) ** 2).sum() / B

    dcount = np.maximum(lens - 1.0, 1.0)
    acount = np.maximum(lens - 2.0, 1.0)
    gt2 = lens_i > 2

    # boundary-junk corrections (device sums include columns touching the
    # zeroed tail; emulate the device's bf16 arithmetic and subtract).
    s_last_b = bf(rec[ar, last, 4])
    d3_last_b = bf(rec[ar, last, 3])
    has_j1 = (lens_i >= 1) & (lens_i <= wb_s - 1)       # col len-1 in W-1 diffs
    decel_junk = np.where(has_j1, np.maximum(-s_last_b, 0.0), 0.0)
    dd_j = -d3_last_b
    u2_j = bf(dd_j - 2.0 * np.round(0.5 * dd_j))
    dsm_junk = np.where(has_j1, (PI * u2_j) ** 2, 0.0)

    # tsm junk: acc columns at l=len-2 (needs 2<=len<=Wb-1) and l=len-1
    # (needs 1<=len<=Wb-2), emulated in device bf16 arithmetic.
    lm1 = np.clip(lens_i - 1, 0, None)
    lm2 = np.clip(lens_i - 2, 0, None)
    p_l1 = bf(rec[ar, lm1, 0:2])                        # p[len-1]
    p_l2 = bf(rec[ar, lm2, 0:2])                        # p[len-2]
    velA = bf(p_l1 - p_l2)                              # vel[len-2]
    accA = bf(-p_l1 - velA)                             # acc[len-2]
    accB = p_l1                                         # acc[len-1]
    amagA = np.sqrt(bf(bf(accA[:, 0] ** 2) + bf(accA[:, 1] ** 2)))
    amagB = np.sqrt(bf(bf(accB[:, 0] ** 2) + bf(accB[:, 1] ** 2)))
    hasA = (lens_i >= 2) & (lens_i <= wb_s - 1)
    hasB = (lens_i >= 1) & (lens_i <= wb_s - 2)
    tsm_junk = np.where(hasA, amagA, 0.0) + np.where(hasB, amagB, 0.0)

    decel = (parts[:, C_DECEL] - decel_junk) / dcount
    s0 = rec[:, 0, 4].astype(np.float64)
    s_last = rec[ar, last, 4].astype(np.float64)
    start_pen = np.maximum(0.3 - s0, 0.0)
    end_pen = np.maximum(s_last - 0.2, 0.0)
    speed_decel_loss = np.where(gt2, decel + 0.5 * (start_pen + end_pen), 0.0).sum() / B

    dir_smooth_loss = np.where(gt2, (parts[:, C_DSM] - dsm_junk) / dcount, 0.0).sum() / B
    traj_smooth_loss = np.where(gt2, (parts[:, C_TSM] - tsm_junk) / acount, 0.0).sum() / B

    # KL from global sums: sum(clip(lv) - exp(clip(lv))) and sum(mu^2)
    kl_loss = -0.5 * (LATENT * B + kl_sums[0] - kl_sums[1]) / B

    total = (sq_term + W_DIR * direction_loss + W_EP * endpoint_loss
             + W_LEN * length_loss + W_DECEL * speed_decel_loss
             + W_DSM * dir_smooth_loss + W_TSM * traj_smooth_loss
             + W_KL * kl_loss)
    return np.float32(total)
